# revision 26
# baseline (speedup 1.0000x reference)
"""Chamfer loss kernel for Trainium2 (8 NeuronCores, data-parallel over batch).

Math: for each batch, d2[m,n] = ||pred_m - gt_n||^2 = p2[m] + g2[n] - 2*dot.
The reference gathers the argmin point and recomputes the distance, which
equals min_n d2[m,n] (resp. min_m), so no argmin/gather is needed:
  fwd_e = sqrt(rowmin(d2) + EPS), bwd_e = sqrt(colmin(d2) + EPS)
  loss = mean(relu(fwd_e - t)) + mean(relu(bwd_e - t))

Device work per core (2 batches): d2 via K=5 fp32 matmul with augmented
operands A = [-2*pred; p2; 1] (lhsT) and B = [gt; 1; g2] (rhs).  Structure is
chosen to minimize the NEFF's *static* instruction stream (measured to
dominate wall time on this backend, ~100us per static matmul instruction,
with loop-body re-execution nearly free): a single For_i hardware loop per
batch over m-tile pairs (G=2), whose body has fully static engine APs — the
only per-iteration dynamic instructions are the weight restage (DVE copy with
register src) and a tiny rowmin writeback (register dst).  DVE reduces
straight out of PSUM in fp32 (no ACT stage, no fp16 slabs): per psum tile a
tensor_tensor MIN accumulate (col-min) and a tensor_reduce MIN (row-min).
GPSIMD collapses the col accumulator across partitions (max of negated).
Host does the tiny epilogue: sqrt/relu/mean on 128K values.
"""

import os
from contextlib import ExitStack

import numpy as np

EPS = 1e-8
B, M, N = 16, 4096, 4096
NCORES = 8
B_LOC = B // NCORES  # batches per core
G = 2  # m-tiles per loop iteration

_CACHE = {}


def build_nc(b_loc=B_LOC, m=M, n=N, reps=1, g=G):
    import concourse.bacc as bacc
    import concourse.mybir as mybir
    import concourse.tile as tile
    from concourse import bass_isa
    from concourse.bass import ds

    f32 = mybir.dt.float32
    f32r = mybir.dt.float32r
    MIN = mybir.AluOpType.min
    E = mybir.EngineType

    nc = bacc.Bacc("TRN2", target_bir_lowering=False, debug=False)
    a_in = nc.dram_tensor("a_in", [b_loc, 5, m], f32, kind="ExternalInput").ap()
    b_in = nc.dram_tensor("b_in", [b_loc, 5, n], f32, kind="ExternalInput").ap()
    n_mt = m // 128
    n_grp = n_mt // g
    # fwd_out[b, p, mt] = min_n d2[mt*128+p, n] = rowmin
    fwd_out = nc.dram_tensor(
        "fwd_out", [b_loc, 128, n_mt], f32, kind="ExternalOutput"
    ).ap()
    # bwd_out[b, 0, n] = -min_m d2[m, n] = -colmin (gpsimd only has max)
    bwd_out = nc.dram_tensor(
        "bwd_out", [b_loc, 1, n], f32, kind="ExternalOutput"
    ).ap()

    hints = (E.PE, E.Activation, E.DVE, E.SP, E.Pool)
    hints_inner = (E.PE, E.Activation, E.DVE)
    with tile.TileContext(nc) as tc, ExitStack() as ctx:
        ab_pool = ctx.enter_context(tc.tile_pool(name="ab", bufs=1))
        ps_pool = ctx.enter_context(tc.tile_pool(name="ps", bufs=2, space="PSUM"))
        cp = ctx.enter_context(tc.tile_pool(name="c", bufs=2))

        for _ in range(reps):
            # outer hardware loop over batches: the per-batch static stream
            # exists once; batch addressing is dynamic only in DMA descriptors
            with tc.For_i(0, b_loc, 1, hint_engines=hints) as b:
                a_sb = ab_pool.tile([5, m], f32, tag="a")
                b_sb = ab_pool.tile([5, n], f32, tag="b")
                nc.sync.dma_start(out=a_sb, in_=a_in[ds(b, 1)])
                nc.sync.dma_start(out=b_sb, in_=b_in[ds(b, 1)])
                # fp32r operands must be rounded by a producer instruction
                b_sbr = ab_pool.tile([5, n], f32r, tag="br")
                nc.vector.tensor_copy(out=b_sbr, in_=b_sb)

                cacc = cp.tile([128, n], f32, tag="cacc")
                fwd = cp.tile([128, n_mt], f32, tag="fwd")
                nc.vector.memset(cacc, 3.0e38)

                with tc.For_i(0, n_grp, 1, hint_engines=hints_inner) as k:
                    wcur = cp.tile([5, g * 128], f32r, tag="wcur")
                    nc.vector.tensor_copy(
                        out=wcur, in_=a_sb[:, ds(k * (g * 128), g * 128)]
                    )
                    rt = cp.tile([128, g, 2], f32, tag="rt")
                    for u in range(g):
                        for h in range(2):
                            ps = ps_pool.tile([128, n // 2], f32, tag="ps")
                            for j in range(n // 2 // 512):
                                n0 = h * (n // 2) + j * 512
                                # float32r streams 1 col/cycle (vs 4 for
                                # fp32) at N>=256 — tf32-like rounding
                                nc.tensor.matmul(
                                    ps[:, j * 512 : (j + 1) * 512],
                                    wcur[:, u * 128 : (u + 1) * 128],
                                    b_sbr[:, n0 : n0 + 512],
                                    start=True,
                                    stop=True,
                                )
                            # col-min accumulate straight from PSUM (fp32)
                            nc.vector.tensor_tensor(
                                out=cacc[:, h * (n // 2) : (h + 1) * (n // 2)],
                                in0=cacc[:, h * (n // 2) : (h + 1) * (n // 2)],
                                in1=ps,
                                op=MIN,
                            )
                            # row-min of this psum tile
                            nc.vector.tensor_reduce(
                                out=rt[:, u, h : h + 1],
                                in_=ps,
                                axis=mybir.AxisListType.X,
                                op=MIN,
                            )
                    # fold the two halves straight into fwd (single register-AP op)
                    nc.vector.tensor_tensor(
                        out=fwd[:, ds(k * g, g)], in0=rt[:, :, 0], in1=rt[:, :, 1], op=MIN
                    )

                # collapse col accumulator across partitions on GPSIMD
                # (only max exists: negate once, host negates back)
                nc.vector.tensor_scalar_mul(cacc, cacc, -1.0)
                pr = cp.tile([128, n], f32, tag="pr")
                nc.gpsimd.partition_all_reduce(
                    pr, cacc, channels=128, reduce_op=bass_isa.ReduceOp.max
                )
                nc.sync.dma_start(out=fwd_out[ds(b, 1)], in_=fwd)
                nc.sync.dma_start(out=bwd_out[ds(b, 1)], in_=pr[0:1, :])
    nc.compile()
    return nc


def _host_prep(predict_pc_6, gt_pc_6):
    """Build augmented matmul operands A (lhsT side) and B (rhs side)."""
    pred = np.ascontiguousarray(predict_pc_6[:, :3, :], dtype=np.float32)
    gt = np.ascontiguousarray(gt_pc_6[:, :3, :], dtype=np.float32)
    A = np.empty((B, 5, M), np.float32)
    A[:, 0:3] = -2.0 * pred
    A[:, 3] = np.einsum("bdm,bdm->bm", pred, pred)
    A[:, 4] = 1.0
    Bm = np.empty((B, 5, N), np.float32)
    Bm[:, 0:3] = gt
    Bm[:, 3] = 1.0
    Bm[:, 4] = np.einsum("bdm,bdm->bm", gt, gt)
    return A, Bm


def kernel(predict_pc_6, gt_pc_6, thresh):
    from concourse.bass_utils import run_bass_kernel_spmd

    predict_pc_6 = np.asarray(predict_pc_6)
    gt_pc_6 = np.asarray(gt_pc_6)
    thresh = np.float32(thresh)

    A, Bm = _host_prep(predict_pc_6, gt_pc_6)

    if "nc" not in _CACHE:
        _CACHE["nc"] = build_nc()
    nc = _CACHE["nc"]

    core_ids = list(range(NCORES))
    in_maps = [
        {
            "a_in": np.ascontiguousarray(A[i * B_LOC : (i + 1) * B_LOC]),
            "b_in": np.ascontiguousarray(Bm[i * B_LOC : (i + 1) * B_LOC]),
        }
        for i in core_ids
    ]
    res = run_bass_kernel_spmd(nc, in_maps, core_ids)
    _CACHE["last_res"] = res

    # Host epilogue on 8 * 2 * (4096 + 4096) values.
    fwd_sum = 0.0
    bwd_sum = 0.0
    for i in core_ids:
        r = res.results[i]
        rowmin = r["fwd_out"].astype(np.float64).reshape(-1)
        colmin = -r["bwd_out"].astype(np.float64).reshape(-1)
        fwd_e = np.sqrt(np.maximum(rowmin, 0.0) + EPS)
        bwd_e = np.sqrt(np.maximum(colmin, 0.0) + EPS)
        fwd_sum += np.maximum(fwd_e - float(thresh), 0.0).sum()
        bwd_sum += np.maximum(bwd_e - float(thresh), 0.0).sum()

    loss = fwd_sum / (B * M) + bwd_sum / (B * N)
    return np.float32(loss)


# revision 28
# speedup vs baseline: 1.5007x; 1.5007x over previous
"""Chamfer loss kernel for Trainium2 (8 NeuronCores, data-parallel over batch).

Math: for each batch, d2[m,n] = ||pred_m - gt_n||^2 = p2[m] + g2[n] - 2*dot.
The reference gathers the argmin point and recomputes the distance, which
equals min_n d2[m,n] (resp. min_m), so no argmin/gather is needed:
  fwd_e = sqrt(rowmin(d2) + EPS), bwd_e = sqrt(colmin(d2) + EPS)
  loss = mean(relu(fwd_e - t)) + mean(relu(bwd_e - t))

Device work per core (2 batches): d2 via K=5 fp32 matmul with augmented
operands A = [-2*pred; p2; 1] (lhsT) and B = [gt; 1; g2] (rhs).  Structure is
chosen to minimize the NEFF's *static* instruction stream (measured to
dominate wall time on this backend, ~100us per static matmul instruction,
with loop-body re-execution nearly free): a single For_i hardware loop per
batch over m-tile pairs (G=2), whose body has fully static engine APs — the
only per-iteration dynamic instructions are the weight restage (DVE copy with
register src) and a tiny rowmin writeback (register dst).  DVE reduces
straight out of PSUM in fp32 (no ACT stage, no fp16 slabs): per psum tile a
tensor_tensor MIN accumulate (col-min) and a tensor_reduce MIN (row-min).
GPSIMD collapses the col accumulator across partitions (max of negated).
Host does the tiny epilogue: sqrt/relu/mean on 128K values.
"""

import os
from contextlib import ExitStack

import numpy as np

EPS = 1e-8
B, M, N = 16, 4096, 4096
NCORES = 8
B_LOC = B // NCORES  # batches per core
G = 2  # m-tiles per loop iteration

_CACHE = {}


def build_nc(b_loc=B_LOC, m=M, n=N, reps=1, g=G):
    import concourse.bacc as bacc
    import concourse.mybir as mybir
    import concourse.tile as tile
    from concourse import bass_isa
    from concourse.bass import ds

    f32 = mybir.dt.float32
    f32r = mybir.dt.float32r
    f16 = mybir.dt.float16
    MIN = mybir.AluOpType.min
    Copy = mybir.ActivationFunctionType.Copy
    E = mybir.EngineType

    nc = bacc.Bacc("TRN2", target_bir_lowering=False, debug=False)
    a_in = nc.dram_tensor("a_in", [b_loc, 5, m], f32, kind="ExternalInput").ap()
    b_in = nc.dram_tensor("b_in", [b_loc, 5, n], f32, kind="ExternalInput").ap()
    n_mt = m // 128
    n_grp = n_mt // g
    # fwd_out[b, p, mt] = min_n d2[mt*128+p, n] = rowmin
    fwd_out = nc.dram_tensor(
        "fwd_out", [b_loc, 128, n_mt], f32, kind="ExternalOutput"
    ).ap()
    # bwd_out[b, 0, n] = -min_m d2[m, n] = -colmin (gpsimd only has max)
    bwd_out = nc.dram_tensor(
        "bwd_out", [b_loc, 1, n], f16, kind="ExternalOutput"
    ).ap()

    hints = (E.PE, E.Activation, E.DVE, E.SP, E.Pool)
    hints_inner = (E.PE, E.Activation, E.DVE)
    with tile.TileContext(nc) as tc, ExitStack() as ctx:
        ab_pool = ctx.enter_context(tc.tile_pool(name="ab", bufs=1))
        ps_pool = ctx.enter_context(tc.tile_pool(name="ps", bufs=2, space="PSUM"))
        cp = ctx.enter_context(tc.tile_pool(name="c", bufs=2))
        slab_pool = ctx.enter_context(tc.tile_pool(name="sl", bufs=2))

        for _ in range(reps):
            # outer hardware loop over batches: the per-batch static stream
            # exists once; batch addressing is dynamic only in DMA descriptors
            with tc.For_i(0, b_loc, 1, hint_engines=hints) as b:
                a_sb = ab_pool.tile([5, m], f32, tag="a")
                b_sb = ab_pool.tile([5, n], f32, tag="b")
                nc.sync.dma_start(out=a_sb, in_=a_in[ds(b, 1)])
                nc.sync.dma_start(out=b_sb, in_=b_in[ds(b, 1)])
                # fp32r operands must be rounded by a producer instruction
                b_sbr = ab_pool.tile([5, n], f32r, tag="br")
                nc.vector.tensor_copy(out=b_sbr, in_=b_sb)

                cacc = cp.tile([128, n], f16, tag="cacc")
                fwd = cp.tile([128, n_mt], f32, tag="fwd")
                nc.vector.memset(cacc, 60000.0)

                with tc.For_i(0, n_grp, 1, hint_engines=hints_inner) as k:
                    wcur = cp.tile([5, g * 128], f32r, tag="wcur")
                    nc.vector.tensor_copy(
                        out=wcur, in_=a_sb[:, ds(k * (g * 128), g * 128)]
                    )
                    rt = cp.tile([128, g, 2], f32, tag="rt")
                    for u in range(g):
                        for h in range(2):
                            ps = ps_pool.tile([128, n // 2], f32, tag="ps")
                            for j in range(n // 2 // 512):
                                n0 = h * (n // 2) + j * 512
                                # float32r streams 1 col/cycle (vs 4 for
                                # fp32) at N>=256 — tf32-like rounding
                                nc.tensor.matmul(
                                    ps[:, j * 512 : (j + 1) * 512],
                                    wcur[:, u * 128 : (u + 1) * 128],
                                    b_sbr[:, n0 : n0 + 512],
                                    start=True,
                                    stop=True,
                                )
                            # ACT (otherwise idle) copies psum to an
                            # fp16 slab; DVE col-min then runs 2x from SBUF
                            slab = slab_pool.tile([128, n // 2], f16, tag="sl")
                            nc.scalar.activation(out=slab, in_=ps, func=Copy)
                            nc.vector.tensor_tensor(
                                out=cacc[:, h * (n // 2) : (h + 1) * (n // 2)],
                                in0=cacc[:, h * (n // 2) : (h + 1) * (n // 2)],
                                in1=slab,
                                op=MIN,
                            )
                            # row-min of this psum tile
                            nc.vector.tensor_reduce(
                                out=rt[:, u, h : h + 1],
                                in_=ps,
                                axis=mybir.AxisListType.X,
                                op=MIN,
                            )
                    # fold the two halves straight into fwd (single register-AP op)
                    nc.vector.tensor_tensor(
                        out=fwd[:, ds(k * g, g)], in0=rt[:, :, 0], in1=rt[:, :, 1], op=MIN
                    )

                # collapse col accumulator across partitions on GPSIMD
                # (only max exists: negate once, host negates back)
                nc.vector.tensor_scalar_mul(cacc, cacc, -1.0)
                pr = cp.tile([128, n], f16, tag="pr")
                nc.gpsimd.partition_all_reduce(
                    pr, cacc, channels=128, reduce_op=bass_isa.ReduceOp.max
                )
                nc.sync.dma_start(out=fwd_out[ds(b, 1)], in_=fwd)
                nc.sync.dma_start(out=bwd_out[ds(b, 1)], in_=pr[0:1, :])
    nc.compile()
    return nc


def _host_prep(predict_pc_6, gt_pc_6):
    """Build augmented matmul operands A (lhsT side) and B (rhs side)."""
    pred = np.ascontiguousarray(predict_pc_6[:, :3, :], dtype=np.float32)
    gt = np.ascontiguousarray(gt_pc_6[:, :3, :], dtype=np.float32)
    A = np.empty((B, 5, M), np.float32)
    A[:, 0:3] = -2.0 * pred
    A[:, 3] = np.einsum("bdm,bdm->bm", pred, pred)
    A[:, 4] = 1.0
    Bm = np.empty((B, 5, N), np.float32)
    Bm[:, 0:3] = gt
    Bm[:, 3] = 1.0
    Bm[:, 4] = np.einsum("bdm,bdm->bm", gt, gt)
    return A, Bm


def kernel(predict_pc_6, gt_pc_6, thresh):
    from concourse.bass_utils import run_bass_kernel_spmd

    predict_pc_6 = np.asarray(predict_pc_6)
    gt_pc_6 = np.asarray(gt_pc_6)
    thresh = np.float32(thresh)

    A, Bm = _host_prep(predict_pc_6, gt_pc_6)

    if "nc" not in _CACHE:
        _CACHE["nc"] = build_nc()
    nc = _CACHE["nc"]

    core_ids = list(range(NCORES))
    in_maps = [
        {
            "a_in": np.ascontiguousarray(A[i * B_LOC : (i + 1) * B_LOC]),
            "b_in": np.ascontiguousarray(Bm[i * B_LOC : (i + 1) * B_LOC]),
        }
        for i in core_ids
    ]
    res = run_bass_kernel_spmd(nc, in_maps, core_ids)
    _CACHE["last_res"] = res

    # Host epilogue on 8 * 2 * (4096 + 4096) values.
    fwd_sum = 0.0
    bwd_sum = 0.0
    for i in core_ids:
        r = res.results[i]
        rowmin = r["fwd_out"].astype(np.float64).reshape(-1)
        colmin = -r["bwd_out"].astype(np.float64).reshape(-1)
        fwd_e = np.sqrt(np.maximum(rowmin, 0.0) + EPS)
        bwd_e = np.sqrt(np.maximum(colmin, 0.0) + EPS)
        fwd_sum += np.maximum(fwd_e - float(thresh), 0.0).sum()
        bwd_sum += np.maximum(bwd_e - float(thresh), 0.0).sum()

    loss = fwd_sum / (B * M) + bwd_sum / (B * N)
    return np.float32(loss)
